# revision 9
# baseline (speedup 1.0000x reference)
"""Trainium2 Bass kernel for a 2-layer GCN (GCNConv -> relu -> GCNConv -> relu -> Linear).

Math: with s = deg^-1/2 (deg over dst incl. self-loops), per-edge norm = s[src]*s[dst]:
  h1 = relu( (A_norm @ x) @ W1 + b1 )     (aggregate 4-wide first - linearity)
  h2 = relu( (A_norm @ h1) @ W2 + b2 )
  out = h2 @ Wf + bf

Device strategy (8 cores, nodes sharded, edges partitioned by dst):
  - edges sorted by (supertile, src_chunk, dst_tile, dst), padded per
    (dst_tile, src_chunk) group to multiples of 128; identical schedule on all cores.
  - per 128-edge chunk: M[k,c] = (dst_local_k == c) * s_dst_k  (fp16,
    tensor_scalar(iota, dstv, sdst, is_equal, mult)); segment-sum on PE:
    psum[F,128dst] += features[128k,F].T @ M[128k,128] accumulated over chunks.
  - layer-1 features: x[src]*s[src] streamed per edge (4-wide, host-gathered x);
    layer-2 features: dma_gather (int16 idx within src chunks) from the AllGather'ed
    h1' table [N, 128] fp16 (64 features zero-padded to 256B rows).
  - evacuation per dst tile feature-major: psum -> W matmul -> relu+bias (ACT) ->
    (L1: PE-transpose -> *s_dst -> fp16 -> h1loc)  (L2: Wf matmul -> +bf -> out_fm).
"""
import numpy as np
from contextlib import ExitStack
from dataclasses import dataclass


@dataclass(frozen=True)
class Cfg:
    n_nodes: int = 100000
    n_cores: int = 8
    f_in: int = 4
    f_hid: int = 64
    f_out: int = 2
    src_chunks: int = 4
    st_tiles: int = 6

    @property
    def shard(self):
        return self.n_nodes // self.n_cores

    @property
    def n_tiles(self):
        return (self.shard + 127) // 128

    @property
    def last_rows(self):
        return self.shard - (self.n_tiles - 1) * 128

    @property
    def src_chunk(self):
        return self.n_nodes // self.src_chunks

    @property
    def n_st(self):
        return (self.n_tiles + self.st_tiles - 1) // self.st_tiles

    def tiles_of_st(self, st):
        return list(range(st * self.st_tiles, min((st + 1) * self.st_tiles, self.n_tiles)))


CFG = Cfg()
P = 128
GATHER_SPLIT = 0       # if >0, max chunks (128 idxs each) per dma_gather
SINGLE_PACKET = False  # single_packet=True caps gathers at ~1024 idxs on HW


def preprocess(cfg, x, edge_index, W1, b1, W2, b2, Wf, bf):
    """Host-side sharding: sort/group/pad edges, build per-core device arrays."""
    F_IN = cfg.f_in
    src = np.asarray(edge_index[0], dtype=np.int64)
    dst = np.asarray(edge_index[1], dtype=np.int64)
    loop = np.arange(cfg.n_nodes, dtype=np.int64)
    src = np.concatenate([src, loop])
    dst = np.concatenate([dst, loop])

    deg = np.bincount(dst, minlength=cfg.n_nodes).astype(np.float64)
    s = (1.0 / np.sqrt(deg)).astype(np.float32)  # deg >= 1 (self-loops)

    core_id = dst // cfg.shard
    x = np.asarray(x, dtype=np.float32)

    # group sequence (st, c, t) shared by all cores
    seq = []
    for st in range(cfg.n_st):
        for c in range(cfg.src_chunks):
            for t in cfg.tiles_of_st(st):
                seq.append((t, c))
    n_grp = len(seq)

    grp_base = np.zeros(cfg.n_st, dtype=np.int64)
    acc = 0
    for st in range(cfg.n_st):
        grp_base[st] = acc
        acc += cfg.src_chunks * len(cfg.tiles_of_st(st))

    per_core = []
    counts = np.zeros((cfg.n_cores, n_grp), dtype=np.int64)
    for cpu in range(cfg.n_cores):
        m = core_id == cpu
        sc, dc = src[m], dst[m]
        dl = dc - cpu * cfg.shard
        tl = dl // P
        ch = sc // cfg.src_chunk
        stl = tl // cfg.st_tiles
        order = np.lexsort((dl, tl, ch, stl))
        sc, dc, dl, tl, ch, stl = (a[order] for a in (sc, dc, dl, tl, ch, stl))
        t_in_st = tl % cfg.st_tiles
        tiles_in_st = np.minimum(cfg.st_tiles, cfg.n_tiles - stl * cfg.st_tiles)
        gseq = grp_base[stl] + ch * tiles_in_st + t_in_st
        counts[cpu] = np.bincount(gseq, minlength=n_grp)
        per_core.append((sc, dc, dl, tl, gseq))

    C = (np.ceil(counts.max(axis=0) / P)).astype(np.int64)
    C = np.maximum(C, 1)
    col_off = np.zeros(n_grp + 1, dtype=np.int64)
    np.cumsum(C, out=col_off[1:])
    NCOLS = int(col_off[-1])
    EPAD = NCOLS * P

    # schedule metadata
    g_i = 0
    st_meta = []
    tile_chunks = [[] for _ in range(cfg.n_tiles)]
    for st in range(cfg.n_st):
        row = []
        for c in range(cfg.src_chunks):
            tiles = cfg.tiles_of_st(st)
            colstart = int(col_off[g_i])
            G = int(sum(C[g_i + k] for k in range(len(tiles))))
            for k, t in enumerate(tiles):
                g = g_i + k
                base = int(col_off[g])
                for j in range(int(C[g])):
                    tile_chunks[t].append((base + j, st, c, base + j - colstart))
            g_i += len(tiles)
            row.append((colstart, G))
        st_meta.append(row)

    dev = []
    for cpu in range(cfg.n_cores):
        sc, dc, dl, tl, gseq = per_core[cpu]
        n = len(sc)
        starts = np.zeros(n_grp, dtype=np.int64)
        starts[1:] = np.cumsum(counts[cpu])[:-1]
        rank = np.arange(n) - starts[gseq]
        pos = col_off[gseq] * P + rank

        idxl = np.zeros(EPAD, dtype=np.int16)
        idxl[pos] = (sc - (sc // cfg.src_chunk) * cfg.src_chunk).astype(np.int16)
        dstv = np.full(EPAD, -1.0, dtype=np.float32)
        dstv[pos] = (dl - tl * P).astype(np.float32)
        sdst = np.zeros(EPAD, dtype=np.float32)
        sdst[pos] = s[dc]
        ssrc = np.zeros(EPAD, dtype=np.float32)
        ssrc[pos] = s[sc]
        xsrc = np.zeros((EPAD, F_IN), dtype=np.float32)
        xsrc[pos] = x[sc]

        stream5 = np.empty((P, NCOLS, F_IN + 1), dtype=np.float32)
        stream5[:, :, :F_IN] = xsrc.reshape(NCOLS, P, F_IN).transpose(1, 0, 2)
        stream5[:, :, F_IN] = ssrc.reshape(NCOLS, P).T
        ds = np.empty((P, NCOLS, 2), dtype=np.float32)
        ds[:, :, 0] = dstv.reshape(NCOLS, P).T
        ds[:, :, 1] = sdst.reshape(NCOLS, P).T
        idx_w = np.tile(idxl.reshape(NCOLS * 8, 16).T, (8, 1))

        s_core = np.zeros(cfg.n_tiles * P, dtype=np.float32)
        s_core[:cfg.shard] = s[cpu * cfg.shard:(cpu + 1) * cfg.shard]
        s_nm = s_core.reshape(cfg.n_tiles, P).T.copy()

        dev.append(dict(stream5=stream5, ds=ds, idx=np.ascontiguousarray(idx_w), s_nm=s_nm))

    wb = dict(
        W1=np.asarray(W1, np.float32), W2=np.asarray(W2, np.float32),
        Wf=np.asarray(Wf, np.float32),
        b1=np.asarray(b1, np.float32).reshape(cfg.f_hid, 1),
        b2=np.asarray(b2, np.float32).reshape(cfg.f_hid, 1),
        bf=np.asarray(bf, np.float32).reshape(cfg.f_out, 1),
    )
    sched = dict(NCOLS=NCOLS, st_meta=st_meta, tile_chunks=tile_chunks)
    return dev, wb, sched


def emulate(cfg, dev, wb, sched):
    """Numpy emulation of the device program (incl. fp16 rounding)."""
    f16 = np.float16
    F_IN, F_HID, F_OUT = cfg.f_in, cfg.f_hid, cfg.f_out
    h1loc_all = []
    for cpu in range(cfg.n_cores):
        d = dev[cpu]
        st5, ds, snm = d["stream5"], d["ds"], d["s_nm"]
        xs16 = (st5[:, :, :F_IN] * st5[:, :, F_IN:F_IN + 1]).astype(f16)
        h1loc = np.zeros((cfg.shard, F_HID), dtype=f16)
        for t in range(cfg.n_tiles):
            acc = np.zeros((F_IN, P), dtype=np.float32)
            for (col, _st, _c, _jj) in sched["tile_chunks"][t]:
                M = (np.equal(np.arange(P)[None, :], ds[:, col, 0:1]) * ds[:, col, 1:2]).astype(f16)
                acc += xs16[:, col, :].astype(np.float32).T @ M.astype(np.float32)
            z = wb["W1"].T @ acc + wb["b1"]
            h1 = np.maximum(z, 0.0)
            rows = cfg.last_rows if t == cfg.n_tiles - 1 else P
            h1n = (h1.T[:rows] * snm[:rows, t:t + 1]).astype(f16)
            h1loc[t * P: t * P + rows] = h1n
        h1loc_all.append(h1loc)
    h1tab = np.concatenate(h1loc_all, axis=0)
    outs = []
    for cpu in range(cfg.n_cores):
        d = dev[cpu]
        ds, idx = d["ds"], d["idx"]
        out_fm = np.zeros((F_OUT, cfg.shard), dtype=np.float32)
        idxl = idx[:16].T.reshape(-1)
        for t in range(cfg.n_tiles):
            acc = np.zeros((F_HID, P), dtype=np.float32)
            for (col, st, c, jj) in sched["tile_chunks"][t]:
                lin = idxl[col * P:(col + 1) * P].astype(np.int64) + c * cfg.src_chunk
                g16 = h1tab[lin]
                M = (np.equal(np.arange(P)[None, :], ds[:, col, 0:1]) * ds[:, col, 1:2]).astype(f16)
                acc += g16.astype(np.float32).T @ M.astype(np.float32)
            z = wb["W2"].T @ acc + wb["b2"]
            h2 = np.maximum(z, 0.0)
            o = wb["Wf"].T @ h2 + wb["bf"]
            rows = cfg.last_rows if t == cfg.n_tiles - 1 else P
            out_fm[:, t * P:t * P + rows] = o[:, :rows]
        outs.append(out_fm)
    return np.concatenate([o.T for o in outs], axis=0)


def build(cfg, sched):
    import concourse.bass as bass
    import concourse.mybir as mybir
    import concourse.tile as tile
    from concourse import bacc

    dt = mybir.dt
    F_IN, F_HID, F_OUT = cfg.f_in, cfg.f_hid, cfg.f_out
    ncols = sched["NCOLS"]
    st_meta = sched["st_meta"]
    tile_chunks = sched["tile_chunks"]

    nc = bacc.Bacc("TRN2", target_bir_lowering=False, num_devices=cfg.n_cores)
    stream5_in = nc.declare_dram_parameter("stream5", [P, ncols, F_IN + 1], dt.float32, isOutput=False)
    ds_in = nc.declare_dram_parameter("ds", [P, ncols, 2], dt.float32, isOutput=False)
    idx_in = nc.declare_dram_parameter("idx", [P, ncols * 8], dt.int16, isOutput=False)
    snm_in = nc.declare_dram_parameter("s_nm", [P, cfg.n_tiles], dt.float32, isOutput=False)
    W1_in = nc.declare_dram_parameter("W1", [F_IN, F_HID], dt.float32, isOutput=False)
    W2_in = nc.declare_dram_parameter("W2", [F_HID, F_HID], dt.float32, isOutput=False)
    Wf_in = nc.declare_dram_parameter("Wf", [F_HID, F_OUT], dt.float32, isOutput=False)
    b1_in = nc.declare_dram_parameter("b1", [F_HID, 1], dt.float32, isOutput=False)
    b2_in = nc.declare_dram_parameter("b2", [F_HID, 1], dt.float32, isOutput=False)
    bf_in = nc.declare_dram_parameter("bf", [F_OUT, 1], dt.float32, isOutput=False)
    out_ext = nc.declare_dram_parameter("out_fm", [F_OUT, cfg.shard], dt.float32, isOutput=True)

    FP = F_HID * 2  # fp16 row padded to 256B

    with tile.TileContext(nc, num_cores=cfg.n_cores) as tc, ExitStack() as ctx:
        dram = ctx.enter_context(tc.tile_pool(name="dram", bufs=1, space="DRAM"))
        const = ctx.enter_context(tc.tile_pool(name="const", bufs=1))
        mpool = ctx.enter_context(tc.tile_pool(name="mpool", bufs=6))
        evac = ctx.enter_context(tc.tile_pool(name="evac", bufs=3))

        h1loc = dram.tile([cfg.shard, FP], dt.float16)
        h1tab = dram.tile([cfg.n_nodes, FP], dt.float16)

        iota_i = const.tile([P, P], dt.int16)
        nc.gpsimd.iota(iota_i[:], pattern=[[1, P]], base=0, channel_multiplier=0)
        iota16 = const.tile([P, P], dt.float16)
        nc.vector.tensor_copy(iota16[:], iota_i[:])
        identi = const.tile([F_HID, F_HID], dt.int16)
        nc.gpsimd.iota(identi[:], pattern=[[1, F_HID]], base=0, channel_multiplier=0)
        identf = const.tile([F_HID, F_HID], dt.float32)
        nc.vector.tensor_copy(identf[:], identi[:])
        iotap = const.tile([F_HID, 1], dt.int16)
        nc.gpsimd.iota(iotap[:], pattern=[[0, 1]], base=0, channel_multiplier=1)
        iotapf = const.tile([F_HID, 1], dt.float32)
        nc.vector.tensor_copy(iotapf[:], iotap[:])
        ident = const.tile([F_HID, F_HID], dt.float32)
        nc.vector.tensor_scalar(out=ident[:], in0=identf[:], scalar1=iotapf[:],
                                scalar2=None, op0=mybir.AluOpType.is_equal)

        W1s = const.tile([F_IN, F_HID], dt.float32)
        W2s = const.tile([F_HID, F_HID], dt.float32)
        Wfs = const.tile([F_HID, F_OUT], dt.float32)
        b1s = const.tile([F_HID, 1], dt.float32)
        b2s = const.tile([F_HID, 1], dt.float32)
        bfs = const.tile([F_OUT, 1], dt.float32)
        snm = const.tile([P, cfg.n_tiles], dt.float32)
        nc.sync.dma_start(W1s[:], W1_in[:])
        nc.sync.dma_start(W2s[:], W2_in[:])
        nc.sync.dma_start(Wfs[:], Wf_in[:])
        nc.sync.dma_start(b1s[:], b1_in[:])
        nc.sync.dma_start(b2s[:], b2_in[:])
        nc.sync.dma_start(bfs[:], bf_in[:])
        nc.sync.dma_start(snm[:], snm_in[:])

        ds_all = const.tile([P, ncols, 2], dt.float32)
        nc.sync.dma_start(ds_all[:], ds_in[:])

        # ---------------- layer 1 ----------------
        with tc.tile_pool(name="l1s", bufs=2) as l1s, \
             tc.tile_pool(name="l1p", bufs=2, space="PSUM") as l1p:
            for st in range(cfg.n_st):
                tiles = cfg.tiles_of_st(st)
                colstart = st_meta[st][0][0]
                colend = st_meta[st][cfg.src_chunks - 1][0] + st_meta[st][cfg.src_chunks - 1][1]
                n_st = colend - colstart

                st5 = l1s.tile([P, n_st, F_IN + 1], dt.float32, tag="st5")
                nc.sync.dma_start(st5[:], stream5_in[:, colstart:colend, :])
                xs32 = l1s.tile([P, n_st, F_IN], dt.float32, tag="xs32")
                nc.vector.tensor_tensor(
                    out=xs32[:],
                    in0=st5[:, :, 0:F_IN],
                    in1=st5[:, :, F_IN:F_IN + 1].to_broadcast([P, n_st, F_IN]),
                    op=mybir.AluOpType.mult,
                )
                xs16 = l1s.tile([P, n_st, F_IN], dt.float16, tag="xs16")
                nc.scalar.activation(xs16[:], xs32[:], mybir.ActivationFunctionType.Copy)

                for t in tiles:
                    chunks = tile_chunks[t]
                    acc = l1p.tile([F_IN, P], dt.float32, tag="acc1")
                    for ci, (col, _st, _c, _jj) in enumerate(chunks):
                        lc = col - colstart
                        M16 = mpool.tile([P, P], dt.float16, tag="M")
                        nc.vector.tensor_scalar(
                            out=M16[:], in0=iota16[:],
                            scalar1=ds_all[:, col, 0:1], scalar2=ds_all[:, col, 1:2],
                            op0=mybir.AluOpType.is_equal, op1=mybir.AluOpType.mult,
                        )
                        nc.tensor.matmul(
                            acc[:], lhsT=xs16[:, lc, :], rhs=M16[:],
                            start=(ci == 0), stop=(ci == len(chunks) - 1),
                        )
                    a1 = evac.tile([F_IN, P], dt.float32, tag="a1")
                    nc.vector.tensor_copy(a1[:], acc[:])
                    ph = l1p.tile([F_HID, P], dt.float32, tag="ph1")
                    nc.tensor.matmul(ph[:], lhsT=W1s[:], rhs=a1[:], start=True, stop=True)
                    h1f = evac.tile([F_HID, P], dt.float32, tag="h1f")
                    nc.scalar.activation(h1f[:], ph[:], mybir.ActivationFunctionType.Relu,
                                         bias=b1s[:, 0:1])
                    pT = l1p.tile([P, F_HID], dt.float32, tag="pT")
                    nc.tensor.transpose(pT[:], h1f[:], ident[:])
                    h1o = evac.tile([P, FP], dt.float16, tag="h1o")
                    nc.vector.tensor_scalar(
                        out=h1o[:, 0:F_HID], in0=pT[:], scalar1=snm[:, t:t + 1],
                        scalar2=None, op0=mybir.AluOpType.mult,
                    )
                    nc.gpsimd.memset(h1o[:, F_HID:FP], 0.0)
                    rows = cfg.last_rows if t == cfg.n_tiles - 1 else P
                    nc.sync.dma_start(h1loc[t * P:t * P + rows, :], h1o[:rows, :])

        # ---------------- all-gather ----------------
        nc.gpsimd.collective_compute(
            "AllGather", mybir.AluOpType.bypass,
            replica_groups=[list(range(cfg.n_cores))],
            ins=[h1loc[:].opt()],
            outs=[h1tab[:].opt()],
        )

        # ---------------- layer 2 ----------------
        with tc.tile_pool(name="l2s", bufs=2) as l2s, \
             tc.tile_pool(name="l2p", bufs=2, space="PSUM") as l2p:
            for st in range(cfg.n_st):
                tiles = cfg.tiles_of_st(st)
                gts = []
                for c in range(cfg.src_chunks):
                    colstart, G = st_meta[st][c]
                    idx_t = l2s.tile([P, G * 8], dt.int16, tag=f"idx{c}")
                    nc.sync.dma_start(idx_t[:], idx_in[:, colstart * 8:(colstart + G) * 8])
                    gt = l2s.tile([P, G, FP], dt.float16, tag=f"gath{c}")
                    kmax = GATHER_SPLIT if GATHER_SPLIT > 0 else G
                    for a in range(0, G, kmax):
                        k = min(kmax, G - a)
                        nc.gpsimd.dma_gather(
                            out_ap=gt[:, a:a + k, :],
                            in_ap=h1tab[c * cfg.src_chunk:(c + 1) * cfg.src_chunk, :],
                            idxs_ap=idx_t[:, a * 8:(a + k) * 8],
                            num_idxs=k * P,
                            num_idxs_reg=k * P,
                            elem_size=FP,
                            single_packet=SINGLE_PACKET,
                        )
                    gts.append(gt)

                for t in tiles:
                    chunks = tile_chunks[t]
                    acc = l2p.tile([F_HID, P], dt.float32, tag="acc2")
                    for ci, (col, _st, c, jj) in enumerate(chunks):
                        M16 = mpool.tile([P, P], dt.float16, tag="M")
                        nc.vector.tensor_scalar(
                            out=M16[:], in0=iota16[:],
                            scalar1=ds_all[:, col, 0:1], scalar2=ds_all[:, col, 1:2],
                            op0=mybir.AluOpType.is_equal, op1=mybir.AluOpType.mult,
                        )
                        nc.tensor.matmul(
                            acc[:], lhsT=gts[c][:, jj, 0:F_HID], rhs=M16[:],
                            start=(ci == 0), stop=(ci == len(chunks) - 1),
                        )
                    a2 = evac.tile([F_HID, P], dt.float32, tag="a2")
                    nc.vector.tensor_copy(a2[:], acc[:])
                    ph2 = l2p.tile([F_HID, P], dt.float32, tag="ph2")
                    nc.tensor.matmul(ph2[:], lhsT=W2s[:], rhs=a2[:], start=True, stop=True)
                    h2f = evac.tile([F_HID, P], dt.float32, tag="h2f")
                    nc.scalar.activation(h2f[:], ph2[:], mybir.ActivationFunctionType.Relu,
                                         bias=b2s[:, 0:1])
                    po = l2p.tile([F_OUT, P], dt.float32, tag="po")
                    nc.tensor.matmul(po[:], lhsT=Wfs[:], rhs=h2f[:], start=True, stop=True)
                    osb = evac.tile([F_OUT, P], dt.float32, tag="osb")
                    nc.scalar.activation(osb[:], po[:], mybir.ActivationFunctionType.Identity,
                                         bias=bfs[:, 0:1])
                    rows = cfg.last_rows if t == cfg.n_tiles - 1 else P
                    nc.sync.dma_start(out_ext[:, t * P:t * P + rows], osb[:, :rows])

    nc.finalize()
    return nc


def make_in_maps(cfg, dev, wb):
    maps = []
    for cpu in range(cfg.n_cores):
        d = dev[cpu]
        maps.append({
            "stream5": d["stream5"], "ds": d["ds"], "idx": d["idx"], "s_nm": d["s_nm"],
            **{k: wb[k] for k in ("W1", "W2", "Wf", "b1", "b2", "bf")},
        })
    return maps


def kernel(x, edge_index, W1, b1, W2, b2, Wf, bf, _trace=False, _tmpdir=None):
    from concourse.bass_utils import run_bass_kernel_spmd

    cfg = CFG
    dev, wb, sched = preprocess(cfg, x, edge_index, W1, b1, W2, b2, Wf, bf)
    nc = build(cfg, sched)
    in_maps = make_in_maps(cfg, dev, wb)
    res = run_bass_kernel_spmd(nc, in_maps, core_ids=list(range(cfg.n_cores)),
                               trace=_trace, tmpdir=_tmpdir)
    out = np.concatenate([res.results[c]["out_fm"].T for c in range(cfg.n_cores)], axis=0)
    kernel._last_results = res
    return out.astype(np.float32)


# revision 42
# speedup vs baseline: 61.2126x; 61.2126x over previous
"""Trainium2 Bass kernel for a 2-layer GCN (GCNConv -> relu -> GCNConv -> relu -> Linear).

Math: with s = deg^-1/2 (deg over dst incl. self-loops), per-edge norm = s[src]*s[dst]:
  h1 = relu( (A_norm @ x) @ W1 + b1 )     (aggregate 4-wide first - linearity)
  h2 = relu( (A_norm @ h1) @ W2 + b2 )
  out = h2 @ Wf + bf

Device strategy (8 cores, nodes sharded, edges partitioned by dst):
  - edges sorted by (supertile, src_chunk, dst_tile, dst), padded per
    (dst_tile, src_chunk) group to multiples of 128; identical schedule on all cores.
  - per 128-edge chunk: M[k,c] = (dst_local_k == c) * s_dst_k  (fp16,
    tensor_scalar(iota, dstv, sdst, is_equal, mult)); segment-sum on PE:
    psum[F,128dst] += features[128k,F].T @ M[128k,128] accumulated over chunks.
  - layer-1 features: x[src]*s[src] streamed per edge (4-wide, host-gathered x);
    layer-2 features: dma_gather (int16 idx within src chunks) from the AllGather'ed
    h1' table [N, 128] fp16 (64 features zero-padded to 256B rows).
  - evacuation per dst tile feature-major: psum -> W matmul -> relu+bias (ACT) ->
    (L1: PE-transpose -> *s_dst -> fp16 -> h1loc)  (L2: Wf matmul -> +bf -> out_fm).
"""
import numpy as np
from contextlib import ExitStack
from dataclasses import dataclass


@dataclass(frozen=True)
class Cfg:
    n_nodes: int = 100000
    n_cores: int = 8
    f_in: int = 4
    f_hid: int = 64
    f_out: int = 2
    src_chunks: int = 4
    st_tiles: int = 4

    @property
    def shard(self):
        return self.n_nodes // self.n_cores

    @property
    def n_tiles(self):
        return (self.shard + 127) // 128

    @property
    def last_rows(self):
        return self.shard - (self.n_tiles - 1) * 128

    @property
    def src_chunk(self):
        return self.n_nodes // self.src_chunks

    @property
    def n_st(self):
        return (self.n_tiles + self.st_tiles - 1) // self.st_tiles

    def tiles_of_st(self, st):
        return list(range(st * self.st_tiles, min((st + 1) * self.st_tiles, self.n_tiles)))


CFG = Cfg()
P = 128
GATHER_SPLIT = 0       # if >0, max chunks (128 idxs each) per dma_gather
SINGLE_PACKET = False  # single_packet=True caps gathers at ~1024 idxs on HW


def preprocess(cfg, x, edge_index, W1, b1, W2, b2, Wf, bf):
    """Host-side sharding: sort/group/pad edges, build per-core device arrays."""
    F_IN = cfg.f_in
    src = np.asarray(edge_index[0], dtype=np.int64)
    dst = np.asarray(edge_index[1], dtype=np.int64)
    # degree includes the implicit self-loops; the loops themselves are handled
    # on-device from resident data (xown/h1keep), not via the edge stream.
    deg = (np.bincount(dst, minlength=cfg.n_nodes) + 1).astype(np.float64)
    s = (1.0 / np.sqrt(deg)).astype(np.float32)

    core_id = dst // cfg.shard
    x = np.asarray(x, dtype=np.float32)
    # Gather table = 2 AllGather halves; half h holds concat over cores of local
    # rows [h*hs,(h+1)*hs), split into 2 idx chunks each. The first half fires
    # mid-layer-1 and its gathers overlap the second AllGather.
    hs = cfg.shard // 2
    cj = hs * cfg.n_cores // 2         # rows per idx chunk
    assert cj <= 32768 and hs * 2 == cfg.shard and cfg.src_chunks == 4

    # group sequence (st, c, t) shared by all cores
    seq = []
    for st in range(cfg.n_st):
        for c in range(cfg.src_chunks):
            for t in cfg.tiles_of_st(st):
                seq.append((t, c))
    n_grp = len(seq)

    grp_base = np.zeros(cfg.n_st, dtype=np.int64)
    acc = 0
    for st in range(cfg.n_st):
        grp_base[st] = acc
        acc += cfg.src_chunks * len(cfg.tiles_of_st(st))

    per_core = []
    counts = np.zeros((cfg.n_cores, n_grp), dtype=np.int64)
    for cpu in range(cfg.n_cores):
        m = core_id == cpu
        sc, dc = src[m], dst[m]
        dl = dc - cpu * cfg.shard
        tl = dl // P
        lr = sc % cfg.shard
        score = sc // cfg.shard
        h = lr // hs
        trow = score * hs + (lr - h * hs)
        ch = h * 2 + trow // cj
        stl = tl // cfg.st_tiles
        order = np.lexsort((dl, tl, ch, stl))
        sc, dc, dl, tl, ch, stl = (a[order] for a in (sc, dc, dl, tl, ch, stl))
        t_in_st = tl % cfg.st_tiles
        tiles_in_st = np.minimum(cfg.st_tiles, cfg.n_tiles - stl * cfg.st_tiles)
        gseq = grp_base[stl] + ch * tiles_in_st + t_in_st
        counts[cpu] = np.bincount(gseq, minlength=n_grp)
        per_core.append((sc, dc, dl, tl, gseq))

    C = (np.ceil(counts.max(axis=0) / P)).astype(np.int64)
    C = np.maximum(C, 1)
    col_off = np.zeros(n_grp + 1, dtype=np.int64)
    np.cumsum(C, out=col_off[1:])
    NCOLS = int(col_off[-1])
    EPAD = NCOLS * P

    # schedule metadata
    g_i = 0
    st_meta = []
    tile_chunks = [[] for _ in range(cfg.n_tiles)]
    for st in range(cfg.n_st):
        row = []
        for c in range(cfg.src_chunks):
            tiles = cfg.tiles_of_st(st)
            colstart = int(col_off[g_i])
            G = int(sum(C[g_i + k] for k in range(len(tiles))))
            for k, t in enumerate(tiles):
                g = g_i + k
                base = int(col_off[g])
                for j in range(int(C[g])):
                    tile_chunks[t].append((base + j, st, c, base + j - colstart))
            g_i += len(tiles)
            row.append((colstart, G))
        st_meta.append(row)

    dev = []
    for cpu in range(cfg.n_cores):
        sc, dc, dl, tl, gseq = per_core[cpu]
        n = len(sc)
        starts = np.zeros(n_grp, dtype=np.int64)
        starts[1:] = np.cumsum(counts[cpu])[:-1]
        rank = np.arange(n) - starts[gseq]
        pos = col_off[gseq] * P + rank

        lr2 = sc % cfg.shard
        score2 = sc // cfg.shard
        h2 = lr2 // hs
        trow2 = score2 * hs + (lr2 - h2 * hs)
        idxl = np.zeros(EPAD, dtype=np.int16)
        idxl[pos] = (trow2 % cj).astype(np.int16)
        dstv = np.full(EPAD, -1.0, dtype=np.float32)
        dstv[pos] = (dl - tl * P).astype(np.float32)
        sdst = np.zeros(EPAD, dtype=np.float32)
        sdst[pos] = s[dc]
        ssrc = np.zeros(EPAD, dtype=np.float32)
        ssrc[pos] = s[sc]
        xsrc = np.zeros((EPAD, F_IN), dtype=np.float32)
        xsrc[pos] = x[sc]

        stream5 = np.empty((P, NCOLS, F_IN + 1), dtype=np.float32)
        stream5[:, :, :F_IN] = xsrc.reshape(NCOLS, P, F_IN).transpose(1, 0, 2)
        stream5[:, :, F_IN] = ssrc.reshape(NCOLS, P).T
        ds = np.empty((P, NCOLS, 3), dtype=np.float32)
        ds[:, :, 0] = dstv.reshape(NCOLS, P).T
        ds[:, :, 1] = sdst.reshape(NCOLS, P).T
        ds[:, :, 2] = -ds[:, :, 1]
        idx_w = np.tile(idxl.reshape(NCOLS * 8, 16).T, (8, 1))

        s_core = np.zeros(cfg.n_tiles * P, dtype=np.float32)
        s_core[:cfg.shard] = s[cpu * cfg.shard:(cpu + 1) * cfg.shard]
        s_nm = s_core.reshape(cfg.n_tiles, P).T.copy()

        x_core = np.zeros((cfg.n_tiles * P, F_IN), dtype=np.float32)
        x_core[:cfg.shard] = x[cpu * cfg.shard:(cpu + 1) * cfg.shard]
        xown = x_core.reshape(cfg.n_tiles, P, F_IN).transpose(1, 0, 2).copy()

        dev.append(dict(stream5=stream5, ds=ds, idx=np.ascontiguousarray(idx_w),
                        s_nm=s_nm, xown=xown))

    wb = dict(
        W1=np.asarray(W1, np.float32), W2=np.asarray(W2, np.float32),
        Wf=np.asarray(Wf, np.float32),
        b1=np.asarray(b1, np.float32).reshape(cfg.f_hid, 1),
        b2=np.asarray(b2, np.float32).reshape(cfg.f_hid, 1),
        bf=np.asarray(bf, np.float32).reshape(cfg.f_out, 1),
    )
    sched = dict(NCOLS=NCOLS, st_meta=st_meta, tile_chunks=tile_chunks)
    return dev, wb, sched


def emulate(cfg, dev, wb, sched):
    """Numpy emulation of the device program (incl. fp16 rounding)."""
    f16 = np.float16
    F_IN, F_HID, F_OUT = cfg.f_in, cfg.f_hid, cfg.f_out
    h1loc_all = []
    for cpu in range(cfg.n_cores):
        d = dev[cpu]
        st5, ds, snm = d["stream5"], d["ds"], d["s_nm"]
        xs16 = (st5[:, :, :F_IN] * st5[:, :, F_IN:F_IN + 1]).astype(f16)
        xown16 = (d["xown"] * snm[:, :, None]).astype(f16)
        h1loc = np.zeros((cfg.shard, F_HID), dtype=f16)
        for t in range(cfg.n_tiles):
            Mself = (np.eye(P) * snm[:, t:t + 1]).astype(f16)
            acc = xown16[:, t, :].astype(np.float32).T @ Mself.astype(np.float32)
            for (col, _st, _c, _jj) in sched["tile_chunks"][t]:
                M = (np.equal(np.arange(P)[None, :], ds[:, col, 0:1]) * ds[:, col, 1:2]).astype(f16)
                acc += xs16[:, col, :].astype(np.float32).T @ M.astype(np.float32)
            z = wb["W1"].T @ acc + wb["b1"]
            h1 = np.maximum(z, 0.0)
            rows = cfg.last_rows if t == cfg.n_tiles - 1 else P
            h1n = (h1.T[:rows] * snm[:rows, t:t + 1]).astype(f16)
            h1loc[t * P: t * P + rows] = h1n
        h1loc_all.append(h1loc)
    hs = cfg.shard // 2
    h1tab = np.concatenate(
        [h1loc_all[c][h * hs:(h + 1) * hs]
         for h in range(2) for c in range(cfg.n_cores)], axis=0)
    h1tab_local = np.concatenate(h1loc_all, axis=0)  # core-major for self lookups
    outs = []
    for cpu in range(cfg.n_cores):
        d = dev[cpu]
        ds, idx = d["ds"], d["idx"]
        out_fm = np.zeros((F_OUT, cfg.shard), dtype=np.float32)
        idxl = idx[:16].T.reshape(-1)
        snm = d["s_nm"]
        base = cpu * cfg.shard
        for t in range(cfg.n_tiles):
            rows0 = min(P, cfg.shard - t * P)
            h1k = np.zeros((P, cfg.f_hid), dtype=f16)
            h1k[:rows0] = h1tab_local[base + t * P: base + t * P + rows0]
            Mself = (np.eye(P) * snm[:, t:t + 1]).astype(f16)
            acc16 = np.zeros((cfg.f_hid, P), dtype=f16)
            for c in range(cfg.src_chunks):
                part = np.zeros((cfg.f_hid, P), dtype=np.float32)
                if c == 0:
                    part += h1k.astype(np.float32).T @ Mself.astype(np.float32)
                for (col, st, cc, jj) in sched["tile_chunks"][t]:
                    if cc != c:
                        continue
                    lin = idxl[col * P:(col + 1) * P].astype(np.int64) + c * cfg.src_chunk
                    g16 = h1tab[lin]
                    M = (np.equal(np.arange(P)[None, :], ds[:, col, 0:1]) * ds[:, col, 1:2]).astype(f16)
                    part += g16.astype(np.float32).T @ M.astype(np.float32)
                acc16 = (acc16.astype(np.float32) + part).astype(f16)
            acc = acc16.astype(np.float32)
            z = wb["W2"].astype(f16).astype(np.float32).T @ acc + wb["b2"]
            h2 = np.maximum(z, 0.0)
            o = wb["Wf"].T @ h2 + wb["bf"]
            rows = cfg.last_rows if t == cfg.n_tiles - 1 else P
            out_fm[:, t * P:t * P + rows] = o[:, :rows]
        outs.append(out_fm)
    return np.concatenate([o.T for o in outs], axis=0)


def build(cfg, sched):
    import concourse.bass as bass
    import concourse.mybir as mybir
    import concourse.tile as tile
    from concourse import bacc

    dt = mybir.dt
    F_IN, F_HID, F_OUT = cfg.f_in, cfg.f_hid, cfg.f_out
    ncols = sched["NCOLS"]
    st_meta = sched["st_meta"]
    tile_chunks = sched["tile_chunks"]

    nc = bacc.Bacc("TRN2", target_bir_lowering=False, num_devices=cfg.n_cores)
    stream5_in = nc.declare_dram_parameter("stream5", [P, ncols, F_IN + 1], dt.float32, isOutput=False)
    ds_in = nc.declare_dram_parameter("ds", [P, ncols, 3], dt.float32, isOutput=False)
    idx_in = nc.declare_dram_parameter("idx", [P, ncols * 8], dt.int16, isOutput=False)
    snm_in = nc.declare_dram_parameter("s_nm", [P, cfg.n_tiles], dt.float32, isOutput=False)
    xown_in = nc.declare_dram_parameter("xown", [P, cfg.n_tiles, F_IN], dt.float32, isOutput=False)
    W1_in = nc.declare_dram_parameter("W1", [F_IN, F_HID], dt.float32, isOutput=False)
    W2_in = nc.declare_dram_parameter("W2", [F_HID, F_HID], dt.float32, isOutput=False)
    Wf_in = nc.declare_dram_parameter("Wf", [F_HID, F_OUT], dt.float32, isOutput=False)
    b1_in = nc.declare_dram_parameter("b1", [F_HID, 1], dt.float32, isOutput=False)
    b2_in = nc.declare_dram_parameter("b2", [F_HID, 1], dt.float32, isOutput=False)
    bf_in = nc.declare_dram_parameter("bf", [F_OUT, 1], dt.float32, isOutput=False)
    out_ext = nc.declare_dram_parameter("out_fm", [F_OUT, cfg.shard], dt.float32, isOutput=True)

    FP = F_HID * 2  # fp16 row padded to 256B

    with tile.TileContext(nc, num_cores=cfg.n_cores) as tc, ExitStack() as ctx:
        dram = ctx.enter_context(tc.tile_pool(name="dram", bufs=1, space="DRAM"))
        const = ctx.enter_context(tc.tile_pool(name="const", bufs=1))
        mpool = ctx.enter_context(tc.tile_pool(name="mpool", bufs=12))
        evac = ctx.enter_context(tc.tile_pool(name="evac", bufs=6))

        h1loc = dram.tile([cfg.shard, FP], dt.float16)
        hrows = cfg.n_nodes // 2
        h1tab0 = dram.tile([hrows, FP], dt.float16, name="h1tab0")
        h1tab1 = dram.tile([hrows, FP], dt.float16, name="h1tab1")

        iota_i = const.tile([P, P], dt.int16)
        nc.gpsimd.iota(iota_i[:], pattern=[[1, P]], base=0, channel_multiplier=0)
        iota16 = const.tile([P, P], dt.float16)
        nc.vector.tensor_copy(iota16[:], iota_i[:])
        identi = const.tile([F_HID, F_HID], dt.int16)
        nc.gpsimd.iota(identi[:], pattern=[[1, F_HID]], base=0, channel_multiplier=0)
        identf = const.tile([F_HID, F_HID], dt.float32)
        nc.vector.tensor_copy(identf[:], identi[:])
        iotap = const.tile([F_HID, 1], dt.int16)
        nc.gpsimd.iota(iotap[:], pattern=[[0, 1]], base=0, channel_multiplier=1)
        iotapf = const.tile([F_HID, 1], dt.float32)
        nc.vector.tensor_copy(iotapf[:], iotap[:])
        iotapP = const.tile([P, 1], dt.int16)
        nc.gpsimd.iota(iotapP[:], pattern=[[0, 1]], base=0, channel_multiplier=1)
        iotapPf = const.tile([P, 1], dt.float32)
        nc.vector.tensor_copy(iotapPf[:], iotapP[:])
        h1keep = const.tile([P, cfg.n_tiles, F_HID], dt.float16)
        ident = const.tile([F_HID, F_HID], dt.float32)
        nc.vector.tensor_scalar(out=ident[:], in0=identf[:], scalar1=iotapf[:],
                                scalar2=None, op0=mybir.AluOpType.is_equal)

        W1s = const.tile([F_IN, F_HID], dt.float32)
        W2s = const.tile([F_HID, F_HID], dt.float32)
        Wfs = const.tile([F_HID, F_OUT], dt.float32)
        b1s = const.tile([F_HID, 1], dt.float32)
        b2s = const.tile([F_HID, 1], dt.float32)
        bfs = const.tile([F_OUT, 1], dt.float32)
        snm = const.tile([P, cfg.n_tiles], dt.float32)
        nc.sync.dma_start(W1s[:], W1_in[:])
        nc.sync.dma_start(W2s[:], W2_in[:])
        nc.sync.dma_start(Wfs[:], Wf_in[:])
        nc.sync.dma_start(b1s[:], b1_in[:])
        nc.sync.dma_start(b2s[:], b2_in[:])
        nc.sync.dma_start(bfs[:], bf_in[:])
        nc.sync.dma_start(snm[:], snm_in[:])

        ds_all = const.tile([P, ncols, 3], dt.float32)
        nc.sync.dma_start(ds_all[:], ds_in[:])
        W2s16 = const.tile([F_HID, F_HID], dt.float16)
        nc.scalar.activation(W2s16[:], W2s[:], mybir.ActivationFunctionType.Copy)

        xown_t = const.tile([P, cfg.n_tiles, F_IN], dt.float32)
        nc.sync.dma_start(xown_t[:], xown_in[:])
        xown16 = const.tile([P, cfg.n_tiles, F_IN], dt.float16)
        nc.vector.tensor_tensor(
            out=xown16[:], in0=xown_t[:],
            in1=snm[:].to_broadcast([P, cfg.n_tiles, F_IN]),
            op=mybir.AluOpType.mult,
        )

        # ---------------- layer 1 ----------------
        with tc.tile_pool(name="l1s", bufs=3) as l1s, \
             tc.tile_pool(name="l1p", bufs=2, space="PSUM") as l1p:
            for st in range(cfg.n_st):
                tiles = cfg.tiles_of_st(st)
                colstart = st_meta[st][0][0]
                colend = st_meta[st][cfg.src_chunks - 1][0] + st_meta[st][cfg.src_chunks - 1][1]
                n_st = colend - colstart

                st5 = l1s.tile([P, n_st, F_IN + 1], dt.float32, tag="st5")
                nc.sync.dma_start(st5[:], stream5_in[:, colstart:colend, :])
                xs16 = l1s.tile([P, n_st, F_IN], dt.float16, tag="xs16")
                nc.vector.tensor_tensor(
                    out=xs16[:],
                    in0=st5[:, :, 0:F_IN],
                    in1=st5[:, :, F_IN:F_IN + 1].to_broadcast([P, n_st, F_IN]),
                    op=mybir.AluOpType.mult,
                )

                for t in tiles:
                    chunks = tile_chunks[t]
                    acc = l1p.tile([F_IN, P], dt.float32, tag="acc1", bufs=3)
                    Ms = mpool.tile([P, P], dt.float16, tag="M")
                    nc.vector.tensor_scalar(
                        out=Ms[:], in0=iota16[:],
                        scalar1=iotapPf[:, 0:1], scalar2=snm[:, t:t + 1],
                        op0=mybir.AluOpType.is_equal, op1=mybir.AluOpType.mult,
                    )
                    nc.tensor.matmul(acc[:], lhsT=xown16[:, t, :], rhs=Ms[:],
                                     start=True, stop=False)
                    for ci, (col, _st, _c, _jj) in enumerate(chunks):
                        lc = col - colstart
                        M16 = mpool.tile([P, P], dt.float16, tag="M")
                        if ci % 7 >= 5:
                            nc.gpsimd.tensor_scalar(
                                out=M16[:], in0=iota16[:],
                                scalar1=ds_all[:, col, 0:1], scalar2=ds_all[:, col, 1:2],
                                op0=mybir.AluOpType.is_equal, op1=mybir.AluOpType.mult,
                            )
                        elif ci % 16 == 6:
                            # M = sdst * relu(1 - |iota - dstv|) on the scalar engine
                            A16 = mpool.tile([P, P], dt.float16, tag="MA")
                            nc.scalar.activation(
                                A16[:], iota16[:], mybir.ActivationFunctionType.Abs,
                                bias=ds_all[:, col, 0:1], scale=-1.0)
                            nc.scalar.activation(
                                M16[:], A16[:], mybir.ActivationFunctionType.Relu,
                                bias=ds_all[:, col, 1:2], scale=ds_all[:, col, 2:3])
                        else:
                            nc.vector.tensor_scalar(
                                out=M16[:], in0=iota16[:],
                                scalar1=ds_all[:, col, 0:1], scalar2=ds_all[:, col, 1:2],
                                op0=mybir.AluOpType.is_equal, op1=mybir.AluOpType.mult,
                            )
                        nc.tensor.matmul(
                            acc[:], lhsT=xs16[:, lc, :], rhs=M16[:],
                            start=False, stop=(ci == len(chunks) - 1),
                        )
                    a1 = evac.tile([F_IN, P], dt.float32, tag="a1")
                    nc.scalar.activation(a1[:], acc[:], mybir.ActivationFunctionType.Copy)
                    ph = l1p.tile([F_HID, P], dt.float32, tag="ph1", bufs=3)
                    nc.tensor.matmul(ph[:], lhsT=W1s[:], rhs=a1[:], start=True, stop=True)
                    h1f = evac.tile([F_HID, P], dt.float32, tag="h1f")
                    nc.scalar.activation(h1f[:], ph[:], mybir.ActivationFunctionType.Relu,
                                         bias=b1s[:, 0:1])
                    pT = l1p.tile([P, F_HID], dt.float32, tag="pT")
                    nc.tensor.transpose(pT[:], h1f[:], ident[:])
                    nc.vector.tensor_scalar(
                        out=h1keep[:, t, :], in0=pT[:], scalar1=snm[:, t:t + 1],
                        scalar2=None, op0=mybir.AluOpType.mult,
                    )
                    rows = cfg.last_rows if t == cfg.n_tiles - 1 else P
                    nc.sync.dma_start(h1loc[t * P:t * P + rows, 0:F_HID],
                                      h1keep[:rows, t, :])

        # ------- all-gather (halved: first half fires mid-layer-1) -------
        hs2 = cfg.shard // 2
        for (a, b), tab in (((0, hs2), h1tab0), ((hs2, 2 * hs2), h1tab1)):
            nc.gpsimd.collective_compute(
                "AllGather", mybir.AluOpType.bypass,
                replica_groups=[list(range(cfg.n_cores))],
                ins=[h1loc[a:b, :].opt()],
                outs=[tab[:].opt()],
            )

        # ---------------- layer 2 ----------------
        # Per-(tile, src-chunk) short PSUM groups folded into per-supertile SBUF
        # accumulators (fp16 between phases); table-piece-k work for every supertile
        # is emitted before any piece-k+1 work so gathers flow during collectives.
        with tc.tile_pool(name="l2s", bufs=2) as l2s, \
             tc.tile_pool(name="accp", bufs=cfg.n_st) as accp, \
             tc.tile_pool(name="l2p", bufs=2, space="PSUM") as l2p:
            accs = {}

            def emit_chunk_group(st, c):
                tiles = cfg.tiles_of_st(st)
                ntp = len(tiles) * P
                if c == 0:
                    acc_st = accp.tile([F_HID, ntp], dt.float16, tag="accS", name=f"accS{st}")
                    accs[st] = acc_st
                colstart, G = st_meta[st][c]
                idx_t = l2s.tile([P, G * 8], dt.int16, tag=f"idx{c}")
                nc.sync.dma_start(idx_t[:], idx_in[:, colstart * 8:(colstart + G) * 8])
                gt = l2s.tile([P, G, FP], dt.float16, tag=f"gath{c}")
                kmax = GATHER_SPLIT if GATHER_SPLIT > 0 else G
                for a in range(0, G, kmax):
                    k = min(kmax, G - a)
                    srcs = {0: h1tab0[0:cfg.src_chunk, :],
                            1: h1tab0[cfg.src_chunk:2 * cfg.src_chunk, :],
                            2: h1tab1[0:cfg.src_chunk, :],
                            3: h1tab1[cfg.src_chunk:2 * cfg.src_chunk, :]}
                    nc.gpsimd.dma_gather(
                        out_ap=gt[:, a:a + k, :],
                        in_ap=srcs[c],
                        idxs_ap=idx_t[:, a * 8:(a + k) * 8],
                        num_idxs=k * P,
                        num_idxs_reg=k * P,
                        elem_size=FP,
                        single_packet=SINGLE_PACKET,
                    )
                for ti, t in enumerate(tiles):
                    chs = [e for e in tile_chunks[t] if e[2] == c]
                    acc = l2p.tile([F_HID, P], dt.float32, tag="accq", bufs=4)
                    if c == 0:
                        Ms = mpool.tile([P, P], dt.float16, tag="M2")
                        nc.vector.tensor_scalar(
                            out=Ms[:], in0=iota16[:],
                            scalar1=iotapPf[:, 0:1], scalar2=snm[:, t:t + 1],
                            op0=mybir.AluOpType.is_equal, op1=mybir.AluOpType.mult,
                        )
                        nc.tensor.matmul(acc[:], lhsT=h1keep[:, t, :], rhs=Ms[:],
                                         start=True, stop=False)
                    for ci, (col, _st, _c, jj) in enumerate(chs):
                        M16 = mpool.tile([P, P], dt.float16, tag="M2")
                        nc.vector.tensor_scalar(
                            out=M16[:], in0=iota16[:],
                            scalar1=ds_all[:, col, 0:1], scalar2=ds_all[:, col, 1:2],
                            op0=mybir.AluOpType.is_equal, op1=mybir.AluOpType.mult,
                        )
                        nc.tensor.matmul(
                            acc[:], lhsT=gt[:, jj, 0:F_HID], rhs=M16[:],
                            start=(ci == 0 and c != 0), stop=(ci == len(chs) - 1),
                        )
                    sl = accs[st][:, ti * P:(ti + 1) * P]
                    if c == 0:
                        nc.scalar.activation(sl, acc[:], mybir.ActivationFunctionType.Copy)
                    else:
                        nc.vector.tensor_add(sl, acc[:], sl)

            def emit_final(st):
                tiles = cfg.tiles_of_st(st)
                for ti, t in enumerate(tiles):
                    sl = accs[st][:, ti * P:(ti + 1) * P]
                    ph2 = l2p.tile([F_HID, P], dt.float32, tag="ph2")
                    nc.tensor.matmul(ph2[:], lhsT=W2s16[:], rhs=sl, start=True, stop=True)
                    h2f = evac.tile([F_HID, P], dt.float32, tag="h2f")
                    nc.scalar.activation(h2f[:], ph2[:], mybir.ActivationFunctionType.Relu,
                                         bias=b2s[:, 0:1])
                    po = l2p.tile([F_OUT, P], dt.float32, tag="po")
                    nc.tensor.matmul(po[:], lhsT=Wfs[:], rhs=h2f[:], start=True, stop=True)
                    osb = evac.tile([F_OUT, P], dt.float32, tag="osb")
                    nc.scalar.activation(osb[:], po[:], mybir.ActivationFunctionType.Identity,
                                         bias=bfs[:, 0:1])
                    rows = cfg.last_rows if t == cfg.n_tiles - 1 else P
                    nc.sync.dma_start(out_ext[:, t * P:t * P + rows], osb[:, :rows])

            for st in range(cfg.n_st):
                emit_chunk_group(st, 0)
                emit_chunk_group(st, 1)
            for st in range(cfg.n_st):
                emit_chunk_group(st, 2)
                emit_chunk_group(st, 3)
                emit_final(st)

    nc.finalize()
    return nc


def make_in_maps(cfg, dev, wb):
    maps = []
    for cpu in range(cfg.n_cores):
        d = dev[cpu]
        maps.append({
            "stream5": d["stream5"], "ds": d["ds"], "idx": d["idx"], "s_nm": d["s_nm"],
            "xown": d["xown"],
            **{k: wb[k] for k in ("W1", "W2", "Wf", "b1", "b2", "bf")},
        })
    return maps


def kernel(x, edge_index, W1, b1, W2, b2, Wf, bf, _trace=False, _tmpdir=None):
    from concourse.bass_utils import run_bass_kernel_spmd

    cfg = CFG
    dev, wb, sched = preprocess(cfg, x, edge_index, W1, b1, W2, b2, Wf, bf)
    nc = build(cfg, sched)
    in_maps = make_in_maps(cfg, dev, wb)
    res = run_bass_kernel_spmd(nc, in_maps, core_ids=list(range(cfg.n_cores)),
                               trace=_trace, tmpdir=_tmpdir)
    out = np.concatenate([res.results[c]["out_fm"].T for c in range(cfg.n_cores)], axis=0)
    kernel._last_results = res
    return out.astype(np.float32)
